# revision 8
# baseline (speedup 1.0000x reference)
"""MultiHeadAttention Bass kernel for Trainium2, 8 NeuronCores.

Problem: B=2, T=S=2048, D=1024, H=16 heads, dk=64, causal mask, fp32.
Reference returns (out[B,T,D], attn_w[B,H,T,S]) with the quirk that `out`
is computed from the raw [B,H,T,dk] -> [B,T,H*dk] reshape (no head
transpose back): out rows [128h,128h+128) of batch b come from head h
with timesteps interleaved 16-way (t = 16*t'' + j//64, d = j%64).

Sharding: core c <- batch b=c//4, heads 4g..4g+3 (g=c%4). Host passes
pre-transposed Q^T/K^T/V^T of the core's batch and the head-slices of
the weights; core writes out rows [512g,512g+512) and attn_w[b,4g:4g+4].

All matmuls run in float32r (full PE rate at free dim >= 256, ~2e-4 max
rel err). Causality: strictly-masked tiles are never computed; their
attn_w region stays zero because the PJRT path donates zero-initialized
output buffers. Diagonal tiles get an additive -1e30 mask on PSUM before
exp. Softmax row-sums come free from a ones-column appended to v (pass
A); pass B applies exp and 1/l in a single ScalarE op via
bias = -ln(l[t]) per partition.
"""
import sys

if "/opt/trn_rl_repo" not in sys.path:
    sys.path.insert(0, "/opt/trn_rl_repo")

from contextlib import ExitStack

import numpy as np

import concourse.bass as bass  # noqa: F401
import concourse.tile as tile
from concourse import bacc, mybir
from concourse import bass_utils

F32R = mybir.dt.float32r
F32 = mybir.dt.float32
EXPF = mybir.ActivationFunctionType.Exp
IDF = mybir.ActivationFunctionType.Identity
LNF = mybir.ActivationFunctionType.Ln
MULT = mybir.AluOpType.mult
ADD = mybir.AluOpType.add

B, T, S, D, H = 2, 2048, 2048, 1024, 16
DK = 64
NCORES = 8
HPC = 4            # heads per core
SCALE = 1.0 / 8.0  # 1/sqrt(dk)
NEG = np.float32(-1e30)
NTA = 256          # pass-A t-chunk width


def _emit(nc, d):
    with tile.TileContext(nc) as tc, ExitStack() as ctx:
        per = ctx.enter_context(tc.tile_pool(name="per", bufs=1))
        big = ctx.enter_context(tc.tile_pool(name="big", bufs=3, space="PSUM"))
        avp = ctx.enter_context(tc.tile_pool(name="avp", bufs=1, space="PSUM"))
        wtup = ctx.enter_context(tc.tile_pool(name="wtup", bufs=2))
        wstp = ctx.enter_context(tc.tile_pool(name="wstp", bufs=2))
        osbp = ctx.enter_context(tc.tile_pool(name="osbp", bufs=1))
        attrp = ctx.enter_context(tc.tile_pool(name="attrp", bufs=1))

        def pload(pool, name, shape, dt):
            t = pool.tile(shape, dt, tag=name, name=name)
            nc.sync.dma_start(t[:], d[name][:])
            return t

        bvh_t = pload(per, "bvh", [64, 4], F32)
        ones65_t = pload(per, "ones65", [65, 64], F32)
        id65_t = pload(per, "id65", [65, 1], F32)
        maskA_t = pload(per, "maskA", [128, 384], F32)
        maskB_t = pload(per, "maskB", [128, 896], F32)

        qT = [per.tile([128, T], F32R, tag=f"qT{p}", name=f"qT{p}")
              for p in range(2)]
        kT = [per.tile([128, S], F32R, tag=f"kT{p}", name=f"kT{p}")
              for p in range(2)]
        v_sb = [per.tile([128, 4 * 65], F32R, tag=f"v{sb}", name=f"v{sb}")
                for sb in range(16)]
        # attU: unnormalized attn^T per head [64 d, T]; normalized+biased
        # in place, then bitcast to f32r for the Wo matmul.
        attU = [per.tile([64, T], F32, tag=f"attU{h}", name=f"attU{h}")
                for h in range(HPC)]
        # l rows live at partition 64 (copied from psum row 64)
        lst = [per.tile([65, T], F32, tag=f"lst{hb}", name=f"lst{hb}")
               for hb in range(2)]
        negln = [per.tile([128, 16], F32, tag=f"nl{h}", name=f"nl{h}")
                 for h in range(HPC)]

        for sb in range(16):
            nc.sync.dma_start(v_sb[sb][:, 64:260:65], d["vones"][:])

        # ---------------- projections (scoped pools) ----------------
        with tc.tile_pool(name="projp", bufs=1) as projp, \
                tc.tile_pool(name="ldp", bufs=1) as ldp:
            wqt_t = pload(projp, "wqt", [128, 2048], F32R)
            wkt_t = pload(projp, "wkt", [128, 2048], F32R)
            wvt_t = pload(projp, "wvt", [128, 2048], F32R)
            bq2_t = pload(projp, "bq2", [128, 2], F32)
            bk2_t = pload(projp, "bk2", [128, 2], F32)

            for src, wt, b2, dst in (
                (d["qt"], wqt_t, bq2_t, qT),
                (d["kt"], wkt_t, bk2_t, kT),
            ):
                for tn in range(4):
                    ins = []
                    for mc in range(8):
                        it = ldp.tile([128, 512], F32R, tag=f"ld{mc}",
                                      name=f"ld{mc}")
                        nc.sync.dma_start(
                            it[:],
                            src[128 * mc:128 * (mc + 1), 512 * tn:512 * (tn + 1)],
                        )
                        ins.append(it)
                    for i in range(2):
                        ps = big.tile([128, 512], F32, tag="big", name="pj")
                        for mc in range(8):
                            nc.tensor.matmul(
                                ps[:],
                                wt[:, 256 * mc + 128 * i:256 * mc + 128 * i + 128],
                                ins[mc][:],
                                start=(mc == 0),
                                stop=(mc == 7),
                            )
                        nc.scalar.activation(
                            dst[i][:, 512 * tn:512 * (tn + 1)], ps[:], IDF,
                            bias=b2[:, i:i + 1],
                        )
            for sq in range(4):
                ins = []
                for mc in range(8):
                    it = ldp.tile([128, 512], F32R, tag=f"ld{mc}", name=f"ld{mc}")
                    nc.sync.dma_start(
                        it[:],
                        d["vt"][128 * mc:128 * (mc + 1), 512 * sq:512 * (sq + 1)],
                    )
                    ins.append(it)
                for sbl in range(4):
                    sb = 4 * sq + sbl
                    ps = big.tile([128, 256], F32, tag="big", name="pv")
                    for mc in range(8):
                        nc.tensor.matmul(
                            ps[:],
                            ins[mc][:, 128 * sbl:128 * (sbl + 1)],
                            wvt_t[:, 256 * mc:256 * (mc + 1)],
                            start=(mc == 0),
                            stop=(mc == 7),
                        )
                    dst3 = v_sb[sb][:, 0:260].rearrange(
                        "p (h x) -> p h x", x=65)[:, :, 0:64]
                    src3 = ps[:].rearrange("p (h x) -> p h x", x=64)
                    nc.vector.tensor_copy(dst3, src3)

        # wot/bob loaded after the proj pools close (space reuse)
        wotp = ctx.enter_context(tc.tile_pool(name="wotp", bufs=1))
        wot_t = pload(wotp, "wot", [64, 16384], F32R)
        bob_t = pload(wotp, "bob", [128, 1024], F32)

        # ---------------- pass A: scores^T -> exp -> AV ----------------
        def passA_tc(p, tc_):
            n_sb = 2 * tc_ + 2  # causal s-blocks
            avps = [avp.tile([128, NTA], F32, tag=f"av{hb}", name=f"av{hb}")
                    for hb in range(2)]
            for g in range((n_sb + 3) // 4):
                sbs = list(range(4 * g, min(4 * g + 4, n_sb)))
                pss = [big.tile([128, 1024], F32, tag="big", name="pss")
                       for _ in range(2)]
                for sb in sbs:
                    col = 256 * (sb % 4)
                    for hb in range(2):
                        nc.tensor.matmul(
                            pss[hb][:, col:col + NTA],
                            kT[p][64 * hb:64 * (hb + 1), 128 * sb:128 * (sb + 1)],
                            qT[p][64 * hb:64 * (hb + 1), NTA * tc_:NTA * (tc_ + 1)],
                            start=(sb % 4 in (0, 2)),  # first touch per bank
                            stop=(sb % 4 in (1, 3) or sb == n_sb - 1),
                            tile_position=(64 * hb, 0),
                        )
                for sb in sbs:  # diagonal blocks: additive -1e30 mask
                    r = sb - 2 * tc_
                    if 0 <= r < 2:
                        col = 256 * (sb % 4)
                        for hb in range(2):
                            nc.vector.tensor_tensor(
                                pss[hb][:, col:col + NTA],
                                pss[hb][:, col:col + NTA],
                                maskA_t[:, 128 * (1 - r):128 * (1 - r) + NTA],
                                ADD,
                            )
                gw = 256 * len(sbs)
                for hb in range(2):
                    wt_ = wtup.tile([128, 1024], F32R, tag="wtu", name="wtu")
                    nc.scalar.activation(
                        wt_[:, 0:gw], pss[hb][:, 0:gw], EXPF, scale=SCALE
                    )
                    for sb in sbs:
                        col = 256 * (sb % 4)
                        nc.tensor.matmul(
                            avps[hb][0:65, :],
                            v_sb[sb][:, 65 * (2 * p + hb):65 * (2 * p + hb) + 65],
                            wt_[:, col:col + NTA],
                            start=(sb == 0),
                            stop=(sb == n_sb - 1),
                        )
            for hb in range(2):
                hc = 2 * p + hb
                nc.vector.tensor_copy(
                    attU[hc][:, NTA * tc_:NTA * (tc_ + 1)], avps[hb][0:64, :]
                )
                nc.vector.tensor_copy(
                    lst[hb][64:65, NTA * tc_:NTA * (tc_ + 1)], avps[hb][64:65, :]
                )

        # ---------------- l-chain + attn normalization ----------------
        attR = {}

        def lchain(hc):
            hb = hc % 2
            attR[hc] = attrp.tile([64, T], F32R, tag=f"attr{hb}",
                                  name=f"attr{hb}")
            lrow = lst[hb][64:65, :]
            # transpose l into columns (before in-place reciprocal)
            lcol = big.tile([128, 16], F32, tag="big", name="lcol")
            for tb in range(16):
                nc.tensor.matmul(
                    lcol[:, tb:tb + 1],
                    lst[hb][64:65, 128 * tb:128 * (tb + 1)],
                    id65_t[64:65, 0:1],
                    is_transpose=True,
                    start=(tb == 0),
                    stop=(tb == 15),
                )
            ln_ps = big.tile([128, 16], F32, tag="big", name="lnp")
            nc.scalar.activation(ln_ps[:], lcol[:], LNF)
            nc.vector.tensor_scalar(negln[hc][:], ln_ps[:], -1.0, None, MULT)
            # reciprocal in place (rb matmul below runs in plain fp32)
            nc.vector.reciprocal(lrow, lrow)
            # normalize attn rows: attU *= 1/l (via PE broadcast), += bv
            for cn in range(2):
                rb = big.tile([128, 1024], F32, tag="big", name="rb")
                for c2 in range(2):
                    nc.tensor.matmul(
                        rb[0:64, 512 * c2:512 * (c2 + 1)],
                        ones65_t[64:65, 0:64],
                        lst[hb][64:65,
                                1024 * cn + 512 * c2:1024 * cn + 512 * (c2 + 1)],
                        start=True,  # separate psum banks
                        stop=True,
                    )
                nc.vector.tensor_tensor(
                    attU[hc][:, 1024 * cn:1024 * (cn + 1)],
                    attU[hc][:, 1024 * cn:1024 * (cn + 1)],
                    rb[0:64, :],
                    MULT,
                )
                nc.vector.tensor_scalar(
                    attR[hc][:, 1024 * cn:1024 * (cn + 1)],
                    attU[hc][:, 1024 * cn:1024 * (cn + 1)],
                    bvh_t[:, hc:hc + 1],
                    None,
                    ADD,
                )

        # ------------- pass B: scores [t,s] -> normalized exp -> HBM -------
        def passB_tb(hc, tb):
            p, hb = hc // 2, hc % 2
            sc_max = tb // 4
            for g in range(sc_max // 2 + 1):
                scs = list(range(2 * g, min(2 * g + 2, sc_max + 1)))
                ps = big.tile([128, 1024], F32, tag="big", name="pb")
                for sc in scs:
                    col = 512 * (sc % 2)
                    nc.tensor.matmul(
                        ps[:, col:col + 512],
                        qT[p][64 * hb:64 * (hb + 1), 128 * tb:128 * (tb + 1)],
                        kT[p][64 * hb:64 * (hb + 1), 512 * sc:512 * (sc + 1)],
                        start=True,  # one bank per sc slot
                        stop=True,
                    )
                if sc_max in scs:
                    rB = tb % 4
                    col = 512 * (sc_max % 2)
                    nc.vector.tensor_tensor(
                        ps[:, col:col + 512],
                        ps[:, col:col + 512],
                        maskB_t[:, 384 - 128 * rB:896 - 128 * rB],
                        ADD,
                    )
                gw = 512 * len(scs)
                w_st = wstp.tile([128, 1024], F32, tag="wst", name="wst")
                nc.scalar.activation(
                    w_st[:, 0:gw], ps[:, 0:gw], EXPF,
                    scale=SCALE, bias=negln[hc][:, tb:tb + 1],
                )
                nc.sync.dma_start(
                    d["aw"][hc, 128 * tb:128 * (tb + 1), 1024 * g:1024 * g + gw],
                    w_st[:, 0:gw],
                )

        # ---------------- Wo projection ----------------
        def woproj(hc):
            attr = attR[hc]
            for ncn in range(2):
                ps = big.tile([128, 512], F32, tag="big", name="wo")
                for jj in range(16):
                    nc.tensor.matmul(
                        ps[:],
                        attr[:, jj::16],
                        wot_t[0:64,
                              1024 * jj + 512 * ncn:1024 * jj + 512 * (ncn + 1)],
                        start=(jj == 0),
                        stop=(jj == 15),
                    )
                ot = osbp.tile([128, 512], F32, tag="osb", name="osb")
                nc.vector.tensor_tensor(
                    ot[:], ps[:], bob_t[:, 512 * ncn:512 * (ncn + 1)], ADD
                )
                nc.sync.dma_start(
                    d["o"][128 * hc:128 * (hc + 1), 512 * ncn:512 * (ncn + 1)],
                    ot[:],
                )

        # ---------------- schedule ----------------
        for tc_ in range(8):
            passA_tc(0, tc_)
        lchain(0)
        lchain(1)
        b_units = [(h, tb) for tb in range(16) for h in (0, 1)]
        bi = 0
        for tc_ in range(8):
            passA_tc(1, tc_)
            for _ in range(4):
                if bi < len(b_units):
                    passB_tb(*b_units[bi])
                    bi += 1
        while bi < len(b_units):
            passB_tb(*b_units[bi])
            bi += 1
        woproj(0)
        woproj(1)
        lchain(2)
        lchain(3)
        for tb in range(16):
            passB_tb(2, tb)
            passB_tb(3, tb)
        woproj(2)
        woproj(3)


_NC_CACHE = {}


def build_nc():
    if "nc" in _NC_CACHE:
        return _NC_CACHE["nc"]
    nc = bacc.Bacc("TRN2", target_bir_lowering=False, debug=False,
                   num_devices=NCORES)
    d = {}
    for name, shape, dt in [
        ("qt", [D, T], F32R), ("kt", [D, S], F32R), ("vt", [D, S], F32R),
        ("wqt", [128, 2048], F32R), ("wkt", [128, 2048], F32R),
        ("wvt", [128, 2048], F32R), ("wot", [64, 16384], F32R),
        ("bq2", [128, 2], F32), ("bk2", [128, 2], F32),
        ("bvh", [64, 4], F32), ("bob", [128, 1024], F32),
        ("vones", [128, 4], F32R), ("ones65", [65, 64], F32),
        ("id65", [65, 1], F32),
        ("maskA", [128, 384], F32), ("maskB", [128, 896], F32),
    ]:
        d[name] = nc.dram_tensor(name, shape, dt, kind="ExternalInput").ap()
    d["o"] = nc.dram_tensor("o", [512, 1024], F32, kind="ExternalOutput").ap()
    d["aw"] = nc.dram_tensor("aw", [HPC, T, S], F32, kind="ExternalOutput").ap()
    _emit(nc, d)
    nc.compile()
    _NC_CACHE["nc"] = nc
    return nc


def build_in_maps(Q, K, V, Wq, bq, Wk, bk, Wv, bv, Wo, bo):
    Q, K, V = (np.asarray(x, np.float32) for x in (Q, K, V))
    Wq, Wk, Wv, Wo = (np.asarray(x, np.float32) for x in (Wq, Wk, Wv, Wo))
    bq, bk, bv, bo = (np.asarray(x, np.float32) for x in (bq, bk, bv, bo))

    QT = [np.ascontiguousarray(Q[b].T) for b in range(B)]
    KT = [np.ascontiguousarray(K[b].T) for b in range(B)]
    VT = [np.ascontiguousarray(V[b].T) for b in range(B)]

    def wslice(W_, g):
        wt = W_.T[:, 256 * g:256 * (g + 1)]  # [1024, 256]
        return np.ascontiguousarray(
            wt.reshape(8, 128, 256).transpose(1, 0, 2).reshape(128, 2048))

    wot = np.ascontiguousarray(
        Wo.T.reshape(16, 64, 1024).transpose(1, 0, 2).reshape(64, 16384))

    sl = np.arange(128)[:, None]
    xa = np.arange(384)[None, :]
    maskA = np.where(sl <= xa - 128, 0.0, NEG).astype(np.float32)
    tlB = np.arange(128)[:, None]
    xb = np.arange(896)[None, :]
    maskB = np.where(xb <= tlB + 384, 0.0, NEG).astype(np.float32)

    in_maps = []
    for c in range(NCORES):
        b, g = c // 4, c % 4
        hsl = slice(256 * g, 256 * (g + 1))
        in_maps.append({
            "qt": QT[b], "kt": KT[b], "vt": VT[b],
            "wqt": wslice(Wq, g), "wkt": wslice(Wk, g), "wvt": wslice(Wv, g),
            "wot": wot,
            "bq2": np.ascontiguousarray(bq[hsl].reshape(2, 128).T),
            "bk2": np.ascontiguousarray(bk[hsl].reshape(2, 128).T),
            "bvh": np.ascontiguousarray(bv[hsl].reshape(4, 64).T),
            "bob": np.broadcast_to(bo, (128, 1024)).copy(),
            "vones": np.ones((128, 4), np.float32),
            "ones65": np.ones((65, 64), np.float32),
            "id65": np.ones((65, 1), np.float32),
            "maskA": maskA, "maskB": maskB,
        })
    return in_maps


def assemble(results):
    out = np.empty((B, T, D), np.float32)
    attn_w = np.empty((B, H, T, S), np.float32)
    for c in range(NCORES):
        b, g = c // 4, c % 4
        out[b, 512 * g:512 * (g + 1), :] = results[c]["o"]
        attn_w[b, 4 * g:4 * (g + 1)] = results[c]["aw"]
    return out, attn_w


def kernel(Q, K, V, mask, Wq, bq, Wk, bk, Wv, bv, Wo, bo):
    nc = build_nc()
    in_maps = build_in_maps(Q, K, V, Wq, bq, Wk, bk, Wv, bv, Wo, bo)
    res = bass_utils.run_bass_kernel_spmd(nc, in_maps,
                                          core_ids=list(range(NCORES)))
    return assemble(res.results)
